# revision 7
# baseline (speedup 1.0000x reference)
"""GATv2 layer on 8 Trainium2 NeuronCores.

Problem (hardcoded): B=4, N=256, D=256, HEADS=8, DH=32, neg_slope=0.2.

    X = (H @ W_lin) split into heads               [B, h, N, 32]
    e = leaky_relu(Xi + Xj, 0.2) . a[h]            [B, h, N, N]
    e += ln(A0 + 1e-8);  e = -inf outside mask
    attn = softmax_j(e);  Y = attn @ X  (heads merged) @ W_out

Sharding: 8 cores = (batch b = core//2) x (head-group g = core%2, 4 heads
each).  Every core computes a full [N, D] partial of Y[b] (its 4 heads'
contribution through W_out rows g*128:(g+1)*128); host sums the two
partials per batch.  SPMD: all cores run the same program on pre-sliced
inputs (no partition-id branching).

Math: with q' = a^T X and leaky(u) = 0.2u + 0.8 relu(u),

    e[h,i,j] = q'_i + q'_j·0.2 + 0.8·R,  R = sum_d a_d relu(x_di + x_dj)

and the EXACT identity relu(u + v) = v + max(u, -v) gives

    R[i,j] = q'_j + M[i,j],   M[i,j] = sum_d a_d max(x_di, -x_dj)

so (dropping softmax-row constants) e ≡ 0.8·M[i,j] + q'_j + bias[i,j].
The pairwise max pass needs NO per-query scalar: st = max(x_i, -x_j) is
emitted as ONE wide DVE scalar_tensor_tensor per (64-query group, half)
using stride-0 broadcast access patterns over a column-duplicated copy
of X^T (xdup) and -X^T — 8 instructions replace the 256 per-query
add+relu ops of the previous design (each carried ~200-300ns of fixed
issue overhead).  M's transpose identity M[j,i] = M[i,j] + q'_j - q'_i
keeps the symmetric-quadrant trick: a quadrant computed as a transpose
just uses a 0.2·q'_j bias instead of 1.0·q'_j.

The d-reduction is a PE matmul with a sliding-window view of a zero-
padded block-diagonal 0.8·a weight matrix, accumulating rows 4c+h for
32 query nodes c into one [128, 512] PSUM tile (unchanged).  The fill
PSUM drain is regrouped to query-major e tiles by column-tiled PE
permutation matmuls (lhsT = identity slice).

Precision: X / max scores / AV inputs are fp16; bias tiles and logits
fp32; exp output bf16 for range safety; PSUM always fp32.  Row sums
ride the ACT exp instructions via accum_out.
"""

import numpy as np

try:
    import concourse.bass as bass
except ImportError:  # pragma: no cover - fallback for bare containers
    import sys

    sys.path.insert(0, "/opt/trn_rl_repo")
    import concourse.bass as bass

import concourse.mybir as mybir
import concourse.tile as tile
from concourse import masks
from concourse.ap import AP
from concourse.bass_utils import run_bass_kernel_spmd

F32 = mybir.dt.float32
F16 = mybir.dt.float16
BF16 = mybir.dt.bfloat16
U8 = mybir.dt.uint8
AF = mybir.ActivationFunctionType
ALU = mybir.AluOpType

N = 256
D = 256
HEADS = 8
DH = 32
HL = 4  # heads per core
P = 128
NCORES = 8


def _split_multiwait(nc, maxw=1):
    """Walrus codegen here rejects instructions with >1 sem wait ("Too many
    sync wait commands", CoreV3GenImpl setupSyncWait).  Tile's kernel-tail
    drain carries one wait per ticked processor; hoist the extras into
    single-wait NoOps on the same engine just before the instruction."""
    import bass_rust

    n = 0
    for f in nc.m.functions:
        for b in f.blocks:
            new, changed = [], False
            for i in b.instructions:
                si = i.sync_info
                ow = list(si.on_wait) if (si is not None and si.on_wait) else []
                if len(ow) > maxw:
                    extra, keep = ow[:-maxw], ow[-maxw:]
                    for w in extra:
                        nop = mybir.InstNoOp(name=f"I-waitsplit-{n}")
                        n += 1
                        nop.engine = i.engine
                        nop.sync_info = bass_rust.SyncInfo(on_wait=[w], on_update=[])
                        new.append(nop)
                    i.sync_info = bass_rust.SyncInfo(
                        on_wait=keep,
                        on_update=list(si.on_update) if si.on_update else [],
                    )
                    changed = True
                new.append(i)
            if changed:
                b.instructions = new


def _view(ap, dims, extra_off=0):
    """AP with the same tensor/partition dim but custom free dims
    [[stride, count], ...] (element strides), offset by extra_off."""
    return AP(ap.tensor, ap.offset + extra_off, [list(ap.ap[0])] + dims)


def build_module():
    nc = bass.Bass("TRN2", target_bir_lowering=False, debug=False)

    hb = nc.dram_tensor("Hb", [N, D], F32, kind="ExternalInput").ap()
    wlg = nc.dram_tensor("WlinG", [D, P], F32, kind="ExternalInput").ap()
    wog = nc.dram_tensor("WoutG", [P, D], F32, kind="ExternalInput").ap()
    ag = nc.dram_tensor("aG", [HL, DH], F32, kind="ExternalInput").ap()
    mask_d = nc.dram_tensor("mask", [N, N], U8, kind="ExternalInput").ap()
    a0_d = nc.dram_tensor("A0", [N, N], F32, kind="ExternalInput").ap()
    out_d = nc.dram_tensor("out", [N, D], F32, kind="ExternalOutput").ap()

    with tile.TileContext(nc) as tc:
        _body(nc, tc, hb, wlg, wog, ag, mask_d, a0_d, out_d)
    return nc


def _body(nc, tc, hb, wlg, wog, ag, mask_d, a0_d, out_d):
    from contextlib import ExitStack

    ctx = ExitStack()
    with ctx:
        const = ctx.enter_context(tc.tile_pool(name="const", bufs=1))
        work = ctx.enter_context(tc.tile_pool(name="work", bufs=3))
        spool = ctx.enter_context(tc.tile_pool(name="spool", bufs=2))
        drpool = ctx.enter_context(tc.tile_pool(name="drpool", bufs=3))
        ps = ctx.enter_context(tc.tile_pool(name="ps", bufs=4, space="PSUM"))
        fillps = ctx.enter_context(tc.tile_pool(name="fillps", bufs=2, space="PSUM"))
        epsp = ctx.enter_context(tc.tile_pool(name="epsp", bufs=1, space="PSUM"))

        # ---------------- setup: loads -------------------------------
        identh = const.tile([P, P], F16, name="identh", tag="identh")
        masks.make_identity(nc, identh[:])
        identb = const.tile([P, P], BF16, name="identb", tag="identb")
        nc.vector.tensor_copy(identb[:], identh[:])

        # X-pipeline inputs first (they gate the PE chain), mask/A0 after;
        # small/odd loads go on the ACT HWDGE ring in parallel.
        hbt = [const.tile([P, D], F32, name=f"hbt{k}", tag=f"hbt{k}") for k in range(2)]
        for k in range(2):
            nc.sync.dma_start(out=hbt[k][:], in_=hb[k * P : (k + 1) * P, :])
        wlt = [const.tile([P, P], F32, name=f"wlt{k}", tag=f"wlt{k}") for k in range(2)]
        for k in range(2):
            nc.sync.dma_start(out=wlt[k][:], in_=wlg[k * P : (k + 1) * P, :])
        mskt = [const.tile([P, N], U8, name=f"mskt{k}", tag=f"mskt{k}") for k in range(2)]
        a0t = [const.tile([P, N], F32, name=f"a0t{k}", tag=f"a0t{k}") for k in range(2)]
        for k in range(2):
            nc.sync.dma_start(out=mskt[k][:], in_=mask_d[k * P : (k + 1) * P, :])
            nc.sync.dma_start(out=a0t[k][:], in_=a0_d[k * P : (k + 1) * P, :])

        ablk = const.tile([P, HL], F32, name="ablk", tag="ablk")
        nc.gpsimd.memset(ablk[:], 0.0)
        for h in range(HL):
            nc.scalar.dma_start(
                out=ablk[h * DH : (h + 1) * DH, h : h + 1],
                in_=ag[h : h + 1, :],
            )
        wot = const.tile([P, D], F32, name="wot", tag="wot")
        nc.scalar.dma_start(out=wot[:], in_=wog[:, :])
        wotb = const.tile([P, D], F16, name="wotb", tag="wotb")
        nc.vector.tensor_copy(wotb[:], wot[:])

        ones_t = const.tile([1, P], F16, name="ones_t", tag="ones_t")
        nc.gpsimd.memset(ones_t[:], 1.0)
        eps_col = const.tile([P, 1], F32, name="eps_col", tag="eps_col")
        nc.gpsimd.memset(eps_col[:], 1e-8)

        # ---------------- X-prep, all fp16 ---------------------------
        # HAM warmup matmuls are interleaved into the X-prep chain's
        # dependency gaps; they keep the PE activity window busy so fills
        # start at high clock.  Kept live via warmz -> ablkh.
        wrm = fillps.tile([P, 2 * N], F32, name="wrm", tag="fill")

        def _warm(n):
            for _ in range(n):
                nc.tensor.matmul(
                    wrm[:, :P], lhsT=identb[:], rhs=identb[:], start=True, stop=True
                )

        _warm(1)
        # fp16 images of H-block rows and W_lin columns
        hbh = [const.tile([P, D], F16, name=f"hbh{k}", tag=f"hbh{k}") for k in range(2)]
        nc.scalar.copy(hbh[0][:], hbt[0][:])
        nc.vector.tensor_copy(hbh[1][:], hbt[1][:])
        wlh = [const.tile([P, P], F16, name=f"wlh{k}", tag=f"wlh{k}") for k in range(2)]
        nc.scalar.copy(wlh[0][:], wlt[0][:])
        nc.vector.tensor_copy(wlh[1][:], wlt[1][:])

        # HT = Hb^T (f16), via PE transposes
        ht = [const.tile([P, N], F16, name=f"ht{k}", tag=f"ht{k}") for k in range(2)]
        for cb in range(2):  # column block of Hb = partition block of HT
            for ib in range(2):
                tp = ps.tile([P, N], F16, name="ps_t", tag="ps_t")
                nc.tensor.transpose(
                    tp[:, :P], hbh[ib][:, cb * P : (cb + 1) * P], identh[:]
                )
                if ib == 0:
                    nc.scalar.copy(ht[cb][:, ib * P : (ib + 1) * P], tp[:, :P])
                else:
                    nc.vector.tensor_copy(ht[cb][:, ib * P : (ib + 1) * P], tp[:, :P])
            _warm(2)

        # X^T directly: xt[c, i] = sum_k Wlin[k, c] * HT[k, i]
        xtb = const.tile([P, N], F16, name="xtb", tag="xtb")
        xtps = ps.tile([P, N], F32, name="ps_xt", tag="ps_t")
        for k in range(2):
            nc.tensor.matmul(
                xtps[:], lhsT=wlh[k][:], rhs=ht[k][:], start=(k == 0), stop=(k == 1)
            )
        nc.vector.tensor_copy(xtb[:], xtps[:])
        _warm(2)

        # negated X^T (key side of the pairwise max)
        negxt = const.tile([P, N], F16, name="negxt", tag="negxt")
        nc.scalar.activation(negxt[:], xtb[:], AF.Copy, bias=0.0, scale=-1.0)

        # X blocks for AV (bf16), via PE transposes of xtb
        xpbb = [const.tile([P, P], BF16, name=f"xpbb{ib}", tag=f"xpbb{ib}") for ib in range(2)]
        for ib in range(2):
            tph = ps.tile([P, N], F16, name="ps_t", tag="ps_t")
            nc.tensor.transpose(tph[:, :P], xtb[:, ib * P : (ib + 1) * P], identh[:])
            nc.vector.tensor_copy(xpbb[ib][:], tph[:, :P])
            _warm(2)

        # Zbig: [128, 192] zeros with 0.8*aG[h] block at rows h*32, col
        # 32+32h; window Zbig[:, 32-c:160-c] as lhsT puts head h's query-c
        # reduction at out partition h*32+c.  Built on GpSimd (off the DVE
        # queue); warmz keeps the warmup matmuls live.
        warmz = const.tile([P, HL], F32, name="warmz", tag="warmz")
        nc.vector.tensor_scalar(
            out=warmz[:], in0=wrm[:, :HL], scalar1=0.0, scalar2=None, op0=ALU.mult
        )
        ablkh = const.tile([P, HL], F16, name="ablkh", tag="ablkh")
        nc.gpsimd.tensor_tensor(out=ablkh[:], in0=ablk[:], in1=warmz[:], op=ALU.add)
        zt = const.tile([P, 192], F16, name="zt", tag="zt")
        nc.gpsimd.memset(zt[:], 0.0)
        nc.gpsimd.tensor_scalar(
            out=zt[:, DH : DH + HL * DH : DH],
            in0=ablkh[:],
            scalar1=0.8,
            scalar2=None,
            op0=ALU.mult,
        )

        # ---------------- q' = a^T X  --------------------------------
        qps = ps.tile([HL, N], F32, name="ps_q", tag="ps_t")
        nc.tensor.matmul(qps[:], lhsT=ablkh[:], rhs=xtb[:], start=True, stop=True)
        qp_sb = const.tile([HL, N], F16, name="qp_sb", tag="qp_sb")
        nc.scalar.copy(qp_sb[:], qps[:])

        # q' broadcast along partitions (q'_j along free), per head
        qrow = [const.tile([1, N], F16, name=f"qrow{h}", tag=f"qrow{h}") for h in range(HL)]
        for h in range(HL):
            nc.sync.dma_start(out=qrow[h][:], in_=qp_sb[h : h + 1, :])
        qpb = [const.tile([P, N], F16, name=f"qpb{h}", tag=f"qpb{h}") for h in range(HL)]
        for h in range(HL):
            qbs = ps.tile([P, N], F32, name="ps_t", tag="ps_t")
            nc.tensor.matmul(
                qbs[:], lhsT=ones_t[:], rhs=qrow[h][:], start=True, stop=True
            )
            if h % 2 == 0:
                nc.scalar.copy(qpb[h][:], qbs[:])
            else:
                nc.vector.tensor_copy(qpb[h][:], qbs[:])
        # 0.8*q'_i as a per-partition column for i in [128,256): phase 1's
        # direct half must drop the same softmax-row constant 0.8*q'_i that
        # the transposed half drops implicitly.
        qpc8 = const.tile([P, HL], F32, name="qpc8", tag="qpc8")
        qpcs = ps.tile([P, HL], F16, name="ps_qc", tag="ps_t")
        nc.tensor.transpose(qpcs[:], qp_sb[:, P:N], identh[0:HL, 0:HL])
        nc.scalar.activation(qpc8[:], qpcs[:], AF.Copy, bias=0.0, scale=0.8)

        # ---------------- mask/A0 bias tiles (early, off hot path) ---
        # mtile = ln(A0+1e-8) inside mask, -60000 outside  (fp32)
        mtile = [const.tile([P, N], F32, name=f"mtile{it}", tag=f"mtile{it}") for it in range(2)]
        for it2 in range(2):
            lna = work.tile([P, N], F32, name="lna", tag="lna")
            nc.scalar.activation(lna[:], a0t[it2][:], AF.Ln, bias=eps_col[:])
            nc.gpsimd.memset(mtile[it2][:], -60000.0)
            nc.vector.copy_predicated(mtile[it2][:], mskt[it2][:], lna[:])
        # direct-quadrant bias: mask + 1.0*q'_j ; transposed-quadrant
        # bias (i-block 1, j<128 only): mask + 0.2*q'_j
        mqA = [
            [const.tile([P, N], F32, name=f"mqA{h}_{it}", tag=f"mqA{h}_{it}") for it in range(2)]
            for h in range(HL)
        ]
        mqB = [const.tile([P, P], F32, name=f"mqB{h}", tag=f"mqB{h}") for h in range(HL)]
        for h in range(HL):
            for it2 in range(2):
                nc.vector.tensor_tensor(
                    out=mqA[h][it2][:], in0=mtile[it2][:], in1=qpb[h][:], op=ALU.add
                )
            nc.vector.scalar_tensor_tensor(
                out=mqB[h][:],
                in0=qpb[h][:, 0:P],
                scalar=0.2,
                in1=mtile[1][:, 0:P],
                op0=ALU.mult,
                op1=ALU.add,
            )

        # ------- pairwise max pass + PE reduce + per-half tail -------
        # Two phases (query halves it=0,1): fills 2it,2it+1 then that
        # half's softmax/AV/projection, so the second half's pairwise pass
        # overlaps the first half's tail work.
        e_raw0r = [
            const.tile([P, P], F16, name=f"e_raw0r_{h}", tag=f"e_raw0r_{h}")
            for h in range(HL)
        ]
        ptc0 = const.tile([P, HL * N], BF16, name="ptc0", tag="ptc0")
        pt1 = [const.tile([P, 2 * N], BF16, name=f"pt1_{p}", tag=f"pt1_{p}") for p in range(2)]
        es1 = [const.tile([P, 2 * N], F32, name=f"es1_{p}", tag=f"es1_{p}") for p in range(2)]
        e3c = const.tile([P, HL * N], F32, name="e3c", tag="e3c")
        rec = [
            [const.tile([P, 1], F32, name=f"rec{h}_{it}", tag=f"rec{h}_{it}") for it in range(2)]
            for h in range(HL)
        ]
        att = [
            [const.tile([P, N], BF16, name=f"att{h}_{jh}", tag=f"att{h}_{jh}") for jh in range(2)]
            for h in range(HL)
        ]
        ytile = [const.tile([P, P], F16, name=f"ytile{ib}", tag=f"ytile{ib}") for ib in range(2)]
        yt = const.tile([P, N], F16, name="yt", tag="yt")

        for it in range(2):
            # Phase it=1 generates only the j>=128 half: the (i>=128, j<128)
            # quadrant comes from phase 0's raw (i<128, j>=128) via the
            # transpose identity (PE-transposed below, 0.2*q'_j bias).
            jw = N if it == 0 else P
            j0 = N - jw
            epsall = epsp.tile([P, HL * jw], F32, name="epsall", tag="eps")
            eps = [epsall[:, h * jw : (h + 1) * jw] for h in range(HL)]
            if it == 1:
                # emitted before this phase's fills so it runs off the
                # tail's critical path
                for h in range(HL):
                    tpe = ps.tile([P, P], F16, name="ps_t", tag="ps_t")
                    nc.tensor.transpose(tpe[:], e_raw0r[h][:], identh[:])
                    nc.vector.tensor_tensor(
                        out=es1[h // 2][:, (h % 2) * N : (h % 2) * N + P],
                        in0=tpe[:],
                        in1=mqB[h][:],
                        op=ALU.add,
                    )
            for G in (2 * it, 2 * it + 1):
                # one wide DVE op per half: st[p, c*2jw + half*jw + j] =
                # max(xt[p, 64G+32*half+c], -xt[p, j0+j]) via stride-0
                # broadcast APs (walrus caps APs at 2 free dims, so the
                # query operand broadcasts with a stride-0 LAST dim; all
                # operands stay in SBUF for the DVE 2x 2-port mode).
                stb = spool.tile([P, 32 * 2 * jw], F16, name="stb", tag="stb")
                for half in range(2):
                    i0 = 64 * G + 32 * half
                    nc.vector.scalar_tensor_tensor(
                        out=_view(stb[:], [[2 * jw, 32], [1, jw]],
                                  extra_off=half * jw),
                        in0=_view(xtb[:], [[1, 32], [0, jw]], extra_off=i0),
                        scalar=0.0,
                        in1=_view(negxt[:], [[0, 32], [1, jw]], extra_off=j0),
                        op0=ALU.add,
                        op1=ALU.max,
                    )
                fps = fillps.tile([P, 2 * jw], F32, name="fill", tag="fill")
                for c in range(32):
                    nc.tensor.matmul(
                        fps[:],
                        lhsT=zt[:, DH - c : 160 - c],
                        rhs=stb[:, c * 2 * jw : (c + 1) * 2 * jw],
                        start=(c == 0),
                        stop=(c == 31),
                    )
                dr = drpool.tile([P, 2 * jw], F16, name="dr", tag="dr")
                nc.scalar.copy(dr[:], fps[:])
                # regroup (h,c)-packed rows into query-major e tiles with
                # column-tiled PE permutation matmuls (lhsT = ident slice)
                for h in range(HL):
                    for half in range(2):
                        r0 = (64 * G + 32 * half) % P
                        nc.tensor.matmul(
                            epsall[r0 : r0 + 32, h * jw : (h + 1) * jw],
                            lhsT=identh[:, h * DH : (h + 1) * DH],
                            rhs=dr[:, half * jw : (half + 1) * jw],
                            start=True,
                            stop=True,
                            tile_position=(0, r0),
                        )

            if it == 0:
                # logits, exp (+fused rowsum), reciprocal
                for h in range(HL):
                    nc.vector.tensor_tensor(
                        out=e3c[:, h * N : (h + 1) * N],
                        in0=eps[h],
                        in1=mqA[h][0][:],
                        op=ALU.add,
                    )
                    # raw right half for phase 1's transposed quadrant
                    nc.scalar.copy(e_raw0r[h][:], epsall[:, h * jw + P : h * jw + N])
                for h in range(HL):
                    den = work.tile([P, 1], F32, name="den", tag="den")
                    nc.scalar.activation(
                        ptc0[:, h * N : (h + 1) * N],
                        e3c[:, h * N : (h + 1) * N],
                        AF.Exp,
                        accum_out=den[:],
                    )
                    nc.vector.reciprocal(rec[h][0][:], den[:])
            else:
                for p in range(2):
                    for h in (2 * p, 2 * p + 1):
                        o = (h % 2) * N
                        nc.vector.scalar_tensor_tensor(
                            out=es1[p][:, o + P : o + N],
                            in0=eps[h],
                            scalar=qpc8[:, h : h + 1],
                            in1=mqA[h][1][:, P:N],
                            op0=ALU.subtract,
                            op1=ALU.add,
                        )
                        den = work.tile([P, 1], F32, name="den", tag="den")
                        nc.scalar.activation(
                            pt1[p][:, o : o + N],
                            es1[p][:, o : o + N],
                            AF.Exp,
                            accum_out=den[:],
                        )
                        nc.vector.reciprocal(rec[h][1][:], den[:])

            # attn^T via PE for this half
            for h in range(HL):
                for jh in range(2):
                    tpb = ps.tile([P, N], BF16, name="ps_t", tag="ps_t")
                    src_pt = (
                        ptc0[:, h * N + jh * P : h * N + (jh + 1) * P]
                        if it == 0
                        else pt1[h // 2][
                            :, (h % 2) * N + jh * P : (h % 2) * N + (jh + 1) * P
                        ]
                    )
                    nc.tensor.transpose(tpb[:, :P], src_pt, identb[:])
                    if it == 0:
                        nc.scalar.copy(att[h][jh][:, it * P : (it + 1) * P], tpb[:, :P])
                    else:
                        nc.vector.tensor_copy(
                            att[h][jh][:, it * P : (it + 1) * P], tpb[:, :P]
                        )

            # AV + 1/den scale for i-block it
            ib = it
            for h in range(HL):
                yps = ps.tile([P, DH], F32, name="ps_y", tag="ps_t")
                for k in range(2):
                    nc.tensor.matmul(
                        yps[:],
                        lhsT=att[h][k][:, ib * P : (ib + 1) * P],
                        rhs=xpbb[k][:, h * DH : (h + 1) * DH],
                        start=(k == 0),
                        stop=(k == 1),
                    )
                nc.vector.tensor_scalar(
                    out=ytile[ib][:, h * DH : (h + 1) * DH],
                    in0=yps[:],
                    scalar1=rec[h][ib][:],
                    scalar2=None,
                    op0=ALU.mult,
                )

            # out rows for this i-block: transpose Y then @ WoutG
            tph = ps.tile([P, N], F16, name="ps_t", tag="ps_t")
            nc.tensor.transpose(tph[:, :P], ytile[ib][:], identh[:])
            nc.scalar.copy(yt[:, ib * P : (ib + 1) * P], tph[:, :P])
            ops_ = ps.tile([P, N], F32, name="ps_t", tag="ps_t")
            nc.tensor.matmul(
                ops_[:],
                lhsT=yt[:, ib * P : (ib + 1) * P],
                rhs=wotb[:],
                start=True,
                stop=True,
            )
            osb = work.tile([P, N], F32, name="osb", tag="osb")
            nc.scalar.copy(osb[:], ops_[:])
            nc.sync.dma_start(out=out_d[ib * P : (ib + 1) * P, :], in_=osb[:])


_NC_CACHE = None


def _get_module():
    global _NC_CACHE
    if _NC_CACHE is None:
        nc = build_module()
        _split_multiwait(nc)  # HW-compile only; breaks CoreSim bookkeeping
        _NC_CACHE = nc
    return _NC_CACHE


def make_in_maps(H, mask, A0, W_lin, a, W_out):
    H = np.ascontiguousarray(np.asarray(H, dtype=np.float32))
    W_lin = np.ascontiguousarray(np.asarray(W_lin, dtype=np.float32))
    W_out = np.ascontiguousarray(np.asarray(W_out, dtype=np.float32))
    a = np.ascontiguousarray(np.asarray(a, dtype=np.float32))
    A0 = np.ascontiguousarray(np.asarray(A0, dtype=np.float32))
    mask_u8 = np.ascontiguousarray(np.asarray(mask).astype(np.uint8))
    in_maps = []
    for c in range(NCORES):
        b, g = divmod(c, 2)
        in_maps.append(
            {
                "Hb": H[b],
                "WlinG": np.ascontiguousarray(W_lin[:, g * P : (g + 1) * P]),
                "WoutG": np.ascontiguousarray(W_out[g * P : (g + 1) * P, :]),
                "aG": np.ascontiguousarray(a[g * HL : (g + 1) * HL, :]),
                "mask": mask_u8,
                "A0": A0,
            }
        )
    return in_maps


def run_raw(H, mask, A0, W_lin, a, W_out, **kw):
    nc = _get_module()
    in_maps = make_in_maps(H, mask, A0, W_lin, a, W_out)
    return run_bass_kernel_spmd(nc, in_maps, list(range(NCORES)), **kw)


def assemble(results):
    parts = [results[c]["out"] for c in range(NCORES)]
    out = np.stack(
        [parts[2 * b].astype(np.float32) + parts[2 * b + 1] for b in range(4)]
    )
    return out.astype(np.float32)


def kernel(H, mask, A0, W_lin, a, W_out):
    res = run_raw(H, mask, A0, W_lin, a, W_out)
    return assemble(res.results)


# revision 17
# speedup vs baseline: 1.2920x; 1.2920x over previous
"""GATv2 layer on 8 Trainium2 NeuronCores.

Problem (hardcoded): B=4, N=256, D=256, HEADS=8, DH=32, neg_slope=0.2.

    X = (H @ W_lin) split into heads               [B, h, N, 32]
    e = leaky_relu(Xi + Xj, 0.2) . a[h]            [B, h, N, N]
    e += ln(A0 + 1e-8);  e = -inf outside mask
    attn = softmax_j(e);  Y = attn @ X  (heads merged) @ W_out

Sharding: 8 cores = (batch b = core//2) x (head-group g = core%2, 4 heads
each).  Every core computes a full [N, D] partial of Y[b] (its 4 heads'
contribution through W_out rows g*128:(g+1)*128); host sums the two
partials per batch.  SPMD: all cores run the same program on pre-sliced
inputs (no partition-id branching).

Math trick: leaky(x) = 0.2*x + 0.8*relu(x), so with q = 0.2 * a^T X:

    e[h,i,j] = 0.8 * sum_d a[h,d]*relu(X[h,d,i]+X[h,d,j]) + q[h,i] + q[h,j]

(q_i drops in the softmax).  The pairwise relu pass packs all 4 local
heads' dims on the 128 SBUF partitions (Xt[(h,d), i]) and is one fused
op per (query, j-half), split DVE/ACT by the per-c engine table: DVE
tensor_scalar (single-src, fp16, step-1 SBUF) auto-engages the 4x DVE
perf mode, ~190ns fixed + 0.26ns/col; ACT relu-with-bias is ~290ns
fixed + 0.83ns/col.  The relu values are symmetric in (i,j), so phase 1
only computes the j>=128 half and PE-transposes phase 0's raw right
half for the rest.

The d-reduction is a PE matmul with a sliding-window view of a zero-
padded block-diagonal 0.8*a weight matrix, accumulating rows 4c+h for
32 query nodes c into one [128, 512] PSUM tile.  The fill PSUM drain is
regrouped to query-major e tiles by column-tiled PE permutation matmuls
(lhsT = identity slice).

Precision: the whole X-prep chain runs in fp16 through the PE (H^T
transposes, X^T = Wlin^T-style matmul, AV inputs); bias tiles fp16,
logits fp32; exp output bf16 for range safety; PSUM always fp32.  Row
sums ride the ACT exp instructions via accum_out.
"""

import numpy as np

try:
    import concourse.bass as bass
except ImportError:  # pragma: no cover - fallback for bare containers
    import sys

    sys.path.insert(0, "/opt/trn_rl_repo")
    import concourse.bass as bass

import concourse.mybir as mybir
import concourse.tile as tile
from concourse import masks
from concourse.ap import AP
from concourse.bass_utils import run_bass_kernel_spmd

F32 = mybir.dt.float32
F16 = mybir.dt.float16
BF16 = mybir.dt.bfloat16
U8 = mybir.dt.uint8
AF = mybir.ActivationFunctionType
ALU = mybir.AluOpType

N = 256
D = 256
HEADS = 8
DH = 32
HL = 4  # heads per core
P = 128
NCORES = 8

# Per-c engine assignment for the pairwise relu pass.  DVE tensor_scalar
# runs in the 4x perf mode (~190ns fixed + 0.26ns/col); ACT relu is
# ~290ns fixed + 0.83ns/col.  GpSimd's shared-port locks stall
# concurrent DVE ops - never use it here.
_ACT_C = {0, 3, 6, 10, 13, 16, 20, 23, 26, 29}


def _gen_engine(c):
    return "act" if c in _ACT_C else "dve"


def _split_multiwait(nc, maxw=1):
    """Walrus codegen here rejects instructions with >1 sem wait ("Too many
    sync wait commands", CoreV3GenImpl setupSyncWait).  Tile's kernel-tail
    drain carries one wait per ticked processor; hoist the extras into
    single-wait NoOps on the same engine just before the instruction."""
    import bass_rust

    n = 0
    for f in nc.m.functions:
        for b in f.blocks:
            new, changed = [], False
            for i in b.instructions:
                si = i.sync_info
                ow = list(si.on_wait) if (si is not None and si.on_wait) else []
                if len(ow) > maxw:
                    extra, keep = ow[:-maxw], ow[-maxw:]
                    for w in extra:
                        nop = mybir.InstNoOp(name=f"I-waitsplit-{n}")
                        n += 1
                        nop.engine = i.engine
                        nop.sync_info = bass_rust.SyncInfo(on_wait=[w], on_update=[])
                        new.append(nop)
                    i.sync_info = bass_rust.SyncInfo(
                        on_wait=keep,
                        on_update=list(si.on_update) if si.on_update else [],
                    )
                    changed = True
                new.append(i)
            if changed:
                b.instructions = new


def _view(ap, dims, extra_off=0):
    """AP with the same tensor/partition dim but custom free dims
    [[stride, count], ...] (element strides), offset by extra_off."""
    return AP(ap.tensor, ap.offset + extra_off, [list(ap.ap[0])] + dims)


def build_module():
    nc = bass.Bass("TRN2", target_bir_lowering=False, debug=False)

    hb = nc.dram_tensor("Hb", [N, D], F32, kind="ExternalInput").ap()
    wlg = nc.dram_tensor("WlinG", [D, P], F32, kind="ExternalInput").ap()
    wog = nc.dram_tensor("WoutG", [P, D], F32, kind="ExternalInput").ap()
    ag = nc.dram_tensor("aG", [HL, DH], F32, kind="ExternalInput").ap()
    mask_d = nc.dram_tensor("mask", [N, N], U8, kind="ExternalInput").ap()
    a0_d = nc.dram_tensor("A0", [N, N], F32, kind="ExternalInput").ap()
    out_d = nc.dram_tensor("out", [N, D], F32, kind="ExternalOutput").ap()

    with tile.TileContext(nc) as tc:
        _body(nc, tc, hb, wlg, wog, ag, mask_d, a0_d, out_d)
    return nc


def _body(nc, tc, hb, wlg, wog, ag, mask_d, a0_d, out_d):
    from contextlib import ExitStack

    ctx = ExitStack()
    with ctx:
        const = ctx.enter_context(tc.tile_pool(name="const", bufs=1))
        work = ctx.enter_context(tc.tile_pool(name="work", bufs=3))
        spool = ctx.enter_context(tc.tile_pool(name="spool", bufs=12))
        drpool = ctx.enter_context(tc.tile_pool(name="drpool", bufs=3))
        ps = ctx.enter_context(tc.tile_pool(name="ps", bufs=4, space="PSUM"))
        fillps = ctx.enter_context(tc.tile_pool(name="fillps", bufs=2, space="PSUM"))
        epsp = ctx.enter_context(tc.tile_pool(name="epsp", bufs=1, space="PSUM"))

        # ---------------- setup: loads -------------------------------
        identh = const.tile([P, P], F16, name="identh", tag="identh")
        masks.make_identity(nc, identh[:])
        identb = const.tile([P, P], BF16, name="identb", tag="identb")
        nc.vector.tensor_copy(identb[:], identh[:])

        # X-pipeline inputs first (they gate the PE chain), mask/A0 after;
        # small/odd loads go on the ACT HWDGE ring in parallel.
        hbt = [const.tile([P, D], F32, name=f"hbt{k}", tag=f"hbt{k}") for k in range(2)]
        for k in range(2):
            nc.sync.dma_start(out=hbt[k][:], in_=hb[k * P : (k + 1) * P, :])
        wlt = [const.tile([P, P], F32, name=f"wlt{k}", tag=f"wlt{k}") for k in range(2)]
        for k in range(2):
            nc.sync.dma_start(out=wlt[k][:], in_=wlg[k * P : (k + 1) * P, :])
        mskt = [const.tile([P, N], U8, name=f"mskt{k}", tag=f"mskt{k}") for k in range(2)]
        a0t = [const.tile([P, N], F32, name=f"a0t{k}", tag=f"a0t{k}") for k in range(2)]
        for k in range(2):
            nc.sync.dma_start(out=mskt[k][:], in_=mask_d[k * P : (k + 1) * P, :])
            nc.sync.dma_start(out=a0t[k][:], in_=a0_d[k * P : (k + 1) * P, :])

        ablk = const.tile([P, HL], F32, name="ablk", tag="ablk")
        nc.gpsimd.memset(ablk[:], 0.0)
        for h in range(HL):
            nc.scalar.dma_start(
                out=ablk[h * DH : (h + 1) * DH, h : h + 1],
                in_=ag[h : h + 1, :],
            )
        wot = const.tile([P, D], F32, name="wot", tag="wot")
        nc.scalar.dma_start(out=wot[:], in_=wog[:, :])
        wotb = const.tile([P, D], F16, name="wotb", tag="wotb")
        nc.vector.tensor_copy(wotb[:], wot[:])

        ones_t = const.tile([1, P], F16, name="ones_t", tag="ones_t")
        nc.gpsimd.memset(ones_t[:], 1.0)
        eps_col = const.tile([P, 1], F32, name="eps_col", tag="eps_col")
        nc.gpsimd.memset(eps_col[:], 1e-8)

        # ---------------- X-prep, all fp16 ---------------------------
        # HAM warmup matmuls are interleaved into the X-prep chain's
        # dependency gaps; they keep the PE activity window busy so fills
        # start at high clock.  Kept live via warmz -> ablkh.
        wrm = fillps.tile([P, 2 * N], F32, name="wrm", tag="fill")

        def _warm(n):
            for _ in range(n):
                nc.tensor.matmul(
                    wrm[:, :P], lhsT=identb[:], rhs=identb[:], start=True, stop=True
                )

        _warm(1)
        # fp16 images of H-block rows and W_lin columns
        hbh = [const.tile([P, D], F16, name=f"hbh{k}", tag=f"hbh{k}") for k in range(2)]
        nc.scalar.copy(hbh[0][:], hbt[0][:])
        nc.vector.tensor_copy(hbh[1][:], hbt[1][:])
        wlh = [const.tile([P, P], F16, name=f"wlh{k}", tag=f"wlh{k}") for k in range(2)]
        nc.scalar.copy(wlh[0][:], wlt[0][:])
        nc.vector.tensor_copy(wlh[1][:], wlt[1][:])

        # HT = Hb^T (f16), via PE transposes
        ht = [const.tile([P, N], F16, name=f"ht{k}", tag=f"ht{k}") for k in range(2)]
        for cb in range(2):  # column block of Hb = partition block of HT
            for ib in range(2):
                tp = ps.tile([P, N], F16, name="ps_t", tag="ps_t")
                nc.tensor.transpose(
                    tp[:, :P], hbh[ib][:, cb * P : (cb + 1) * P], identh[:]
                )
                if ib == 0:
                    nc.scalar.copy(ht[cb][:, ib * P : (ib + 1) * P], tp[:, :P])
                else:
                    nc.vector.tensor_copy(ht[cb][:, ib * P : (ib + 1) * P], tp[:, :P])
            _warm(2)

        # X^T directly: xt[c, i] = sum_k Wlin[k, c] * HT[k, i]
        xtb = const.tile([P, N], F16, name="xtb", tag="xtb")
        xtps = ps.tile([P, N], F32, name="ps_xt", tag="ps_t")
        for k in range(2):
            nc.tensor.matmul(
                xtps[:], lhsT=wlh[k][:], rhs=ht[k][:], start=(k == 0), stop=(k == 1)
            )
        nc.vector.tensor_copy(xtb[:], xtps[:])
        _warm(2)

        # fp32 image of X^T: per-partition scalar operands (DVE scalar1 /
        # ACT bias) must be fp32; values identical to the fp16 xtb.
        xtf = const.tile([P, N], F32, name="xtf", tag="xtf")
        nc.vector.tensor_copy(xtf[:], xtb[:])

        # X blocks for AV (bf16), via PE transposes of xtb
        xpbb = [const.tile([P, P], BF16, name=f"xpbb{ib}", tag=f"xpbb{ib}") for ib in range(2)]
        for ib in range(2):
            tph = ps.tile([P, N], F16, name="ps_t", tag="ps_t")
            nc.tensor.transpose(tph[:, :P], xtb[:, ib * P : (ib + 1) * P], identh[:])
            nc.vector.tensor_copy(xpbb[ib][:], tph[:, :P])
            _warm(2)

        # Zbig: [128, 192] zeros with 0.8*aG[h] block at rows h*32, col
        # 32+32h; window Zbig[:, 32-c:160-c] as lhsT puts head h's query-c
        # reduction at out partition h*32+c.  Built on GpSimd (off the DVE
        # queue); warmz keeps the warmup matmuls live.
        warmz = const.tile([P, HL], F32, name="warmz", tag="warmz")
        nc.vector.tensor_scalar(
            out=warmz[:], in0=wrm[:, :HL], scalar1=0.0, scalar2=None, op0=ALU.mult
        )
        ablkh = const.tile([P, HL], F16, name="ablkh", tag="ablkh")
        nc.gpsimd.tensor_tensor(out=ablkh[:], in0=ablk[:], in1=warmz[:], op=ALU.add)
        zt = const.tile([P, 192], F16, name="zt", tag="zt")
        nc.gpsimd.memset(zt[:], 0.0)
        nc.gpsimd.tensor_scalar(
            out=zt[:, DH : DH + HL * DH : DH],
            in0=ablkh[:],
            scalar1=0.8,
            scalar2=None,
            op0=ALU.mult,
        )

        # ---------------- q = 0.2 * a^T X  ---------------------------
        qps = ps.tile([HL, N], F32, name="ps_q", tag="ps_t")
        nc.tensor.matmul(qps[:], lhsT=ablkh[:], rhs=xtb[:], start=True, stop=True)
        qp_sb = const.tile([HL, N], F16, name="qp_sb", tag="qp_sb")
        nc.scalar.activation(qp_sb[:], qps[:], AF.Copy, bias=0.0, scale=0.2)

        # q' broadcast along partitions (q'_j along free), per head
        qrow = [const.tile([1, N], F16, name=f"qrow{h}", tag=f"qrow{h}") for h in range(HL)]
        for h in range(HL):
            nc.sync.dma_start(out=qrow[h][:], in_=qp_sb[h : h + 1, :])
        qpb = [const.tile([P, N], F16, name=f"qpb{h}", tag=f"qpb{h}") for h in range(HL)]
        for h in range(HL):
            qbs = ps.tile([P, N], F32, name="ps_t", tag="ps_t")
            nc.tensor.matmul(
                qbs[:], lhsT=ones_t[:], rhs=qrow[h][:], start=True, stop=True
            )
            if h % 2 == 0:
                nc.scalar.copy(qpb[h][:], qbs[:])
            else:
                nc.vector.tensor_copy(qpb[h][:], qbs[:])

        # ---------------- mask/A0 bias tiles (early, off hot path) ---
        # mtile = ln(A0+1e-8) inside mask, -60000 outside  (f16); then
        # mq[h][it2] = full bias for i-block it2 (mask + q_j), f16 so it
        # can ride PE bias matmuls as rhs.
        mtile = [const.tile([P, N], F16, name=f"mtile{it}", tag=f"mtile{it}") for it in range(2)]
        for it2 in range(2):
            lna = work.tile([P, N], F16, name="lna", tag="lna")
            nc.scalar.activation(lna[:], a0t[it2][:], AF.Ln, bias=eps_col[:])
            nc.gpsimd.memset(mtile[it2][:], -60000.0)
            nc.vector.copy_predicated(mtile[it2][:], mskt[it2][:], lna[:])
        mq = [
            [const.tile([P, N], F16, name=f"mq{h}_{it}", tag=f"mq{h}_{it}") for it in range(2)]
            for h in range(HL)
        ]
        for h in range(HL):
            for it2 in range(2):
                nc.vector.tensor_tensor(
                    out=mq[h][it2][:], in0=mtile[it2][:], in1=qpb[h][:], op=ALU.add
                )

        # ------- pairwise max pass + PE reduce + per-half tail -------
        # Two phases (query halves it=0,1): fills 2it,2it+1 then that
        # half's softmax/AV/projection, so the second half's pairwise pass
        # overlaps the first half's tail work.
        e_raw0r = [
            const.tile([P, P], F16, name=f"e_raw0r_{h}", tag=f"e_raw0r_{h}")
            for h in range(HL)
        ]
        ptc0 = const.tile([P, HL * N], BF16, name="ptc0", tag="ptc0")
        pt1 = [const.tile([P, 2 * N], BF16, name=f"pt1_{p}", tag=f"pt1_{p}") for p in range(2)]
        es1 = [const.tile([P, 2 * N], F32, name=f"es1_{p}", tag=f"es1_{p}") for p in range(2)]
        e3c = const.tile([P, HL * N], F32, name="e3c", tag="e3c")
        rec = [
            [const.tile([P, 1], F32, name=f"rec{h}_{it}", tag=f"rec{h}_{it}") for it in range(2)]
            for h in range(HL)
        ]
        att = [
            [const.tile([P, N], BF16, name=f"att{h}_{jh}", tag=f"att{h}_{jh}") for jh in range(2)]
            for h in range(HL)
        ]
        ytile = [const.tile([P, P], F16, name=f"ytile{ib}", tag=f"ytile{ib}") for ib in range(2)]
        yt = const.tile([P, N], F16, name="yt", tag="yt")

        for it in range(2):
            # Phase it=1 generates only the j>=128 half: the (i>=128, j<128)
            # quadrant comes from phase 0's raw (i<128, j>=128) via the
            # transpose identity (PE-transposed below, 0.2*q'_j bias).
            jw = N if it == 0 else P
            j0 = N - jw
            epsall = epsp.tile([P, HL * jw], F32, name="epsall", tag="eps")
            eps = [epsall[:, h * jw : (h + 1) * jw] for h in range(HL)]
            if it == 1:
                # (i>=128, j<128) quadrant = transpose of phase 0's raw
                # (i<128, j>=128); emitted before this phase's fills so it
                # runs off the tail's critical path
                for h in range(HL):
                    tpe = ps.tile([P, P], F16, name="ps_t", tag="ps_t")
                    nc.tensor.transpose(tpe[:], e_raw0r[h][:], identh[:])
                    nc.vector.tensor_tensor(
                        out=es1[h // 2][:, (h % 2) * N : (h % 2) * N + P],
                        in0=tpe[:],
                        in1=mq[h][1][:, 0:P],
                        op=ALU.add,
                    )
            for G in (2 * it, 2 * it + 1):
                fps = fillps.tile([P, 2 * jw], F32, name="fill", tag="fill")
                for c in range(32):
                    st = spool.tile([P, 2 * jw], F16, name="st", tag="st")
                    for half in range(2):
                        i = 64 * G + 32 * half + c
                        dst = st[:, half * jw : (half + 1) * jw]
                        if _gen_engine(c) == "act":
                            nc.scalar.activation(
                                dst, xtb[:, j0:N], AF.Relu, bias=xtf[:, i : i + 1]
                            )
                        else:
                            nc.vector.tensor_scalar(
                                out=dst,
                                in0=xtb[:, j0:N],
                                scalar1=xtf[:, i : i + 1],
                                scalar2=0.0,
                                op0=ALU.add,
                                op1=ALU.max,
                            )
                    nc.tensor.matmul(
                        fps[:],
                        lhsT=zt[:, DH - c : 160 - c],
                        rhs=st[:],
                        start=(c == 0),
                        stop=(c == 31),
                    )
                dr = drpool.tile([P, 2 * jw], F16, name="dr", tag="dr")
                nc.scalar.copy(dr[:], fps[:])
                # regroup (h,c)-packed rows into query-major e tiles with
                # column-tiled PE permutation matmuls (lhsT = ident slice)
                for h in range(HL):
                    for half in range(2):
                        r0 = (64 * G + 32 * half) % P
                        nc.tensor.matmul(
                            epsall[r0 : r0 + 32, h * jw : (h + 1) * jw],
                            lhsT=identh[:, h * DH : (h + 1) * DH],
                            rhs=dr[:, half * jw : (half + 1) * jw],
                            start=True,
                            stop=True,
                            tile_position=(0, r0),
                        )
                if it == 0:
                    # raw right half for phase 1's transposed quadrant —
                    # copied per-G so it runs off the tail critical path
                    rr = 64 * (G % 2)
                    for h in range(HL):
                        nc.scalar.copy(
                            e_raw0r[h][rr : rr + 64, :],
                            epsall[rr : rr + 64, h * jw + P : h * jw + N],
                        )

            if it == 0:
                # logits (bias add), exp (+fused rowsum), reciprocal
                for h in range(HL):
                    nc.vector.tensor_tensor(
                        out=e3c[:, h * N : (h + 1) * N],
                        in0=eps[h],
                        in1=mq[h][0][:],
                        op=ALU.add,
                    )
                for h in range(HL):
                    den = work.tile([P, 1], F32, name="den", tag="den")
                    nc.scalar.activation(
                        ptc0[:, h * N : (h + 1) * N],
                        e3c[:, h * N : (h + 1) * N],
                        AF.Exp,
                        accum_out=den[:],
                    )
                    nc.vector.reciprocal(rec[h][0][:], den[:])
            else:
                for p in range(2):
                    for h in (2 * p, 2 * p + 1):
                        o = (h % 2) * N
                        nc.vector.tensor_tensor(
                            out=es1[p][:, o + P : o + N],
                            in0=eps[h],
                            in1=mq[h][1][:, P:N],
                            op=ALU.add,
                        )
                        den = work.tile([P, 1], F32, name="den", tag="den")
                        nc.scalar.activation(
                            pt1[p][:, o : o + N],
                            es1[p][:, o : o + N],
                            AF.Exp,
                            accum_out=den[:],
                        )
                        nc.vector.reciprocal(rec[h][1][:], den[:])

            # attn^T via PE for this half
            for h in range(HL):
                for jh in range(2):
                    tpb = ps.tile([P, N], BF16, name="ps_t", tag="ps_t")
                    src_pt = (
                        ptc0[:, h * N + jh * P : h * N + (jh + 1) * P]
                        if it == 0
                        else pt1[h // 2][
                            :, (h % 2) * N + jh * P : (h % 2) * N + (jh + 1) * P
                        ]
                    )
                    nc.tensor.transpose(tpb[:, :P], src_pt, identb[:])
                    if it == 0:
                        nc.scalar.copy(att[h][jh][:, it * P : (it + 1) * P], tpb[:, :P])
                    else:
                        nc.vector.tensor_copy(
                            att[h][jh][:, it * P : (it + 1) * P], tpb[:, :P]
                        )

            # AV + 1/den scale for i-block it
            ib = it
            for h in range(HL):
                yps = ps.tile([P, DH], F32, name="ps_y", tag="ps_t")
                for k in range(2):
                    nc.tensor.matmul(
                        yps[:],
                        lhsT=att[h][k][:, ib * P : (ib + 1) * P],
                        rhs=xpbb[k][:, h * DH : (h + 1) * DH],
                        start=(k == 0),
                        stop=(k == 1),
                    )
                nc.vector.tensor_scalar(
                    out=ytile[ib][:, h * DH : (h + 1) * DH],
                    in0=yps[:],
                    scalar1=rec[h][ib][:],
                    scalar2=None,
                    op0=ALU.mult,
                )

            # out rows for this i-block: transpose Y then @ WoutG
            tph = ps.tile([P, N], F16, name="ps_t", tag="ps_t")
            nc.tensor.transpose(tph[:, :P], ytile[ib][:], identh[:])
            nc.scalar.copy(yt[:, ib * P : (ib + 1) * P], tph[:, :P])
            ops_ = ps.tile([P, N], F32, name="ps_t", tag="ps_t")
            nc.tensor.matmul(
                ops_[:],
                lhsT=yt[:, ib * P : (ib + 1) * P],
                rhs=wotb[:],
                start=True,
                stop=True,
            )
            osb = work.tile([P, N], F32, name="osb", tag="osb")
            nc.scalar.copy(osb[:], ops_[:])
            nc.sync.dma_start(out=out_d[ib * P : (ib + 1) * P, :], in_=osb[:])


_NC_CACHE = None


def _get_module():
    global _NC_CACHE
    if _NC_CACHE is None:
        nc = build_module()
        _split_multiwait(nc)  # HW-compile only; breaks CoreSim bookkeeping
        _NC_CACHE = nc
    return _NC_CACHE


def make_in_maps(H, mask, A0, W_lin, a, W_out):
    H = np.ascontiguousarray(np.asarray(H, dtype=np.float32))
    W_lin = np.ascontiguousarray(np.asarray(W_lin, dtype=np.float32))
    W_out = np.ascontiguousarray(np.asarray(W_out, dtype=np.float32))
    a = np.ascontiguousarray(np.asarray(a, dtype=np.float32))
    A0 = np.ascontiguousarray(np.asarray(A0, dtype=np.float32))
    mask_u8 = np.ascontiguousarray(np.asarray(mask).astype(np.uint8))
    in_maps = []
    for c in range(NCORES):
        b, g = divmod(c, 2)
        in_maps.append(
            {
                "Hb": H[b],
                "WlinG": np.ascontiguousarray(W_lin[:, g * P : (g + 1) * P]),
                "WoutG": np.ascontiguousarray(W_out[g * P : (g + 1) * P, :]),
                "aG": np.ascontiguousarray(a[g * HL : (g + 1) * HL, :]),
                "mask": mask_u8,
                "A0": A0,
            }
        )
    return in_maps


def run_raw(H, mask, A0, W_lin, a, W_out, **kw):
    nc = _get_module()
    in_maps = make_in_maps(H, mask, A0, W_lin, a, W_out)
    return run_bass_kernel_spmd(nc, in_maps, list(range(NCORES)), **kw)


def assemble(results):
    parts = [results[c]["out"] for c in range(NCORES)]
    out = np.stack(
        [parts[2 * b].astype(np.float32) + parts[2 * b + 1] for b in range(4)]
    )
    return out.astype(np.float32)


def kernel(H, mask, A0, W_lin, a, W_out):
    res = run_raw(H, mask, A0, W_lin, a, W_out)
    return assemble(res.results)
